# revision 26
# baseline (speedup 1.0000x reference)
"""Trainium2 Bass kernel for BackendQueryPooling.

Math simplifications (exact unless noted):
  - k-projection folds into the shared query:
        scores[l,h] = x[l] . qw[h],  qw[h] = (q_h @ wk_head_h) / sqrt(hd)
    (bk is constant per h and cancels in softmax).
  - v never materialized: ctx[n,h,:] = (sum_l w[n,h,l] x[l]) @ wvT_head
    since sum_l w = 1 after normalization (bv is zero here; asserted).
  - Unnormalized exp + ones-column normalization in the same matmul.
  - Host-side token compaction: tokens with mask=0 have softmax weight
    exactly 0 in the reference (logit -1e9 underflows), so they are
    dropped. Remaining tokens are SORTED by backend_id so each backend
    owns a contiguous, 128-aligned segment; padding slots have xe rows
    (incl. the ones column) set to 0 so they contribute nothing.
  - Empty backends produce 0 via the has_tokens factor folded into the
    final scale, as the reference zeroes by has_tokens.

Sharding: data-parallel over batch. 16 batches / 8 cores = 2 per core.
Each batch gets a [64, 257] PSUM accumulator (partition n*8+h); its
whole tail (normalize/ctx/out_proj/LN/store) runs as soon as that
batch's chunks finish, so only the last batch's tail is exposed.
y matmuls use zero-padded [128, 32] E blocks so PE tile positions stay
32-aligned.
"""

import sys

sys.path.insert(0, "/opt/trn_rl_repo")

import numpy as np
import ml_dtypes

import concourse.bass as bass
import concourse.bacc as bacc
import concourse.tile as tile
from concourse import mybir
from concourse.bass_utils import run_bass_kernel_spmd

BF16 = ml_dtypes.bfloat16
FP8 = ml_dtypes.float8_e4m3
QW_SHIFT = 13  # qw values ~7e-4 underflow fp8; pre-scale by 2**13, undo in exp scale
F32 = np.float32

B, L, D = 16, 8192, 256
H, HD, NB = 8, 32, 8
NCORES = 8
BPC = B // NCORES          # batches per core
SCALE = 1.0 / np.sqrt(HD)
JW = D + 1                 # 257: xe row width incl. ones column
NG = BPC * NB              # 16 (batch, backend) groups per core

_CACHE = {}
LAST_RESULT = None


def _patched_act_tables():
    """Make the act-table chooser land Exp AND Ln in one set
    (natural_log_exp_and_others) instead of thrashing between
    exp_and_others and natural_log (1.28us reload per switch)."""
    from concourse.hw_specs import get_activation_tables

    AF = mybir.ActivationFunctionType

    def patched(arch):
        t = {k: set(v) for k, v in get_activation_tables(arch).items()}
        for name in t:
            if name != "natural_log_exp_and_others":
                t[name].discard(AF.Exp)
                t[name].discard(AF.Ln)
        return t

    return patched


def _build_nc(K, uniform_ln):
    """K = chunks (of 128 tokens) per (batch, backend) segment.

    uniform_ln: ln_gamma is a uniform vector and ln_beta is zero, so the
    gamma*has_tokens factor collapses into the per-row rstd scale and the
    whole LN runs on the Vector engine (rstd via pow(-0.5))."""
    nc = bacc.Bacc("TRN2", target_bir_lowering=False)
    dt = mybir.dt

    NCHB = NB * K              # l-chunks per batch
    NCH = BPC * NCHB           # l-chunks per core (both batches)
    LP = NCHB * 128            # padded tokens per batch
    HC = NCHB // 2             # chunks per xT half-tile

    # xe pieces (in chunks): coarse early (few DMA triggers), fine at the
    # very end so the last piece's compute tail is short
    def split_pieces(n, final):
        h = n // 2
        if not final or h <= 8:
            return [h, n - h]
        return [h, n - h - 8, 4, 4]

    PIECES = []                # (batch, chunk_off_in_batch, n_chunks)
    for bi in range(BPC):
        off = 0
        for pcs in split_pieces(NCHB, final=(bi == BPC - 1)):
            PIECES.append((bi, off, pcs))
            off += pcs

    xT_d = nc.dram_tensor("xT", [BPC, 2, 128, LP], dt.float8e4, kind="ExternalInput")
    xe_d = nc.dram_tensor("xe", [BPC, 128, NCHB * JW], dt.bfloat16, kind="ExternalInput")
    qwT_d = nc.dram_tensor("qwT", [2, 128, H], dt.float8e4, kind="ExternalInput")
    wvT_d = nc.dram_tensor("wvT", [2, 128, D], dt.bfloat16, kind="ExternalInput")
    woT_d = nc.dram_tensor("woT", [2, 128, D], dt.bfloat16, kind="ExternalInput")
    hg_d = nc.dram_tensor("hg", [4, 2 * BPC], dt.float32, kind="ExternalInput")
    if not uniform_ln:
        gb_d = nc.dram_tensor("gb", [4, 2 * 2 * BPC * D], dt.float32, kind="ExternalInput")
    id_d = nc.dram_tensor("ident", [128, 32], dt.float32, kind="ExternalInput")
    out_d = nc.dram_tensor("out", [NG, D], dt.float32, kind="ExternalOutput")

    with tile.TileContext(nc) as tc:
        with (
            tc.tile_pool(name="consts", bufs=1) as consts,
            tc.tile_pool(name="big", bufs=1) as big,
            tc.tile_pool(name="work", bufs=2) as work,
            tc.tile_pool(name="psc", bufs=3, space="PSUM") as psc,
            tc.tile_pool(name="psy", bufs=1, space="PSUM") as psy,
            tc.tile_pool(name="pst", bufs=2, space="PSUM") as pst,
        ):
            # scores-path const first: it gates the first compute
            qwT_sb = consts.tile([128, 2, H], dt.float8e4)
            nc.scalar.dma_start(out=qwT_sb, in_=qwT_d[:].rearrange("c p h -> p c h"))

            # x loads on the Sync trigger queue; xT in halves so scores can
            # start before the full batch lands
            xT_sb = {}
            xe_sb = []

            def load_xT(bi):
                for hf in range(2):
                    t = big.tile([128, 2, LP // 2], dt.float8e4, tag=f"xT{bi}{hf}")
                    nc.sync.dma_start(
                        out=t,
                        in_=xT_d[bi, :, :, hf * (LP // 2):(hf + 1) * (LP // 2)]
                        .rearrange("c p l -> p c l"),
                    )
                    xT_sb[(bi, hf)] = t

            def load_xe(bi, off, pcs):
                t = big.tile([128, pcs, JW], dt.bfloat16, tag=f"xe{len(xe_sb)}")
                nc.sync.dma_start(
                    out=t,
                    in_=xe_d[bi, :, off * JW:(off + pcs) * JW].rearrange(
                        "p (c j) -> p c j", j=JW
                    ),
                )
                xe_sb.append(t)

            for bi in range(BPC):
                load_xT(bi)
            for bi, off, pcs in PIECES:
                load_xe(bi, off, pcs)

            # tail consts on the Scalar queue
            wvT_sb = consts.tile([128, 2, D], dt.bfloat16)
            nc.scalar.dma_start(out=wvT_sb, in_=wvT_d[:].rearrange("c p e -> p c e"))
            woT_sb = consts.tile([128, 2, D], dt.bfloat16)
            nc.scalar.dma_start(out=woT_sb, in_=woT_d[:].rearrange("c p e -> p c e"))
            id_sb = consts.tile([128, 32], dt.float32)
            nc.scalar.dma_start(out=id_sb, in_=id_d[:])
            hg_sb = consts.tile([4, 2 * BPC], dt.float32)
            nc.scalar.dma_start(out=hg_sb, in_=hg_d[:])
            eps_sb = consts.tile([128, 1], dt.float32)
            nc.vector.memset(eps_sb, 1e-5)
            if not uniform_ln:
                gb_sb = consts.tile([4, 2, 2 * BPC, D], dt.float32)
                nc.scalar.dma_start(
                    out=gb_sb,
                    in_=gb_d[:].rearrange("p (t b e) -> p t b e", t=2, b=2 * BPC),
                )

            # zero-padded E blocks: E5[:, c, g, :] nonzero only at g = nb%4
            E5 = consts.tile([128, NCH, 4, H], dt.bfloat16)
            nc.gpsimd.memset(E5, 0.0)

            y_all = psy.tile([128, JW], dt.float32, tag="y")
            y_ps = [y_all[64 * bi_:64 * (bi_ + 1), :] for bi_ in range(BPC)]

            def tail(g):
                """Normalize/ctx/out_proj/LN/store for one 32-row group
                (4 backends of one batch): y rows 32g..32g+32, out rows
                4g..4g+4."""
                pg = slice(32 * g, 32 * (g + 1))
                s_sb = work.tile([128, 1], dt.float32, tag=f"s{g}")
                nc.vector.tensor_scalar_add(s_sb[pg], y_all[pg, D:JW], 1e-30)
                r_sb = work.tile([128, 1], dt.float32, tag=f"r{g}")
                nc.vector.reciprocal(r_sb[pg], s_sb[pg])
                yn_sb = work.tile([128, D], dt.float32, tag=f"yn{g}")
                nc.vector.tensor_scalar_mul(yn_sb[pg], y_all[pg, 0:D], r_sb[pg])

                # transpose y_norm -> yT [e', dc, (n4 h)]
                yT_sb = work.tile([128, 2, 32], dt.bfloat16, tag=f"yT{g}")
                for ec in range(2):
                    tr_ps = pst.tile([128, 32], dt.float32, tag="tail")
                    nc.tensor.transpose(
                        tr_ps, yn_sb[pg, ec * 128:(ec + 1) * 128], id_sb[pg, :],
                        tile_position=(32 * g, 0),
                    )
                    nc.vector.tensor_copy(yT_sb[:, ec, :], tr_ps)

                # ctxT[e', (t, n4)] via per-head matmuls
                cx_ps = pst.tile([128, 2 * 4], dt.float32, tag="tail")
                yTv = yT_sb.rearrange("p c (n h2) -> p c n h2", h2=H)
                for h in range(H):
                    t, r4 = divmod(h, 4)
                    r0 = r4 * 32
                    for dc in range(2):
                        nc.tensor.matmul(
                            cx_ps[r0:r0 + 32, t * 4:(t + 1) * 4],
                            lhsT=wvT_sb[:, dc, HD * h:HD * h + HD],
                            rhs=yTv[:, dc, :, h],
                            start=(dc == 0),
                            stop=(dc == 1),
                            tile_position=(0, r0),
                        )
                cx_sb = work.tile([128, 2 * 4], dt.bfloat16, tag=f"cxs{g}")
                nc.vector.tensor_copy(cx_sb, cx_ps)

                # out_proj + LayerNorm
                op_ps = pst.tile([4, D], dt.float32, tag="tail")
                for t in range(2):
                    nc.tensor.matmul(
                        op_ps,
                        lhsT=cx_sb[:, t * 4:(t + 1) * 4],
                        rhs=woT_sb[:, t, :],
                        start=(t == 0),
                        stop=(t == 1),
                    )
                st_sb = work.tile([4, 6], dt.float32, tag=f"st{g}")
                nc.vector.bn_stats(st_sb, op_ps)
                mv_sb = work.tile([4, 2], dt.float32, tag=f"mv{g}")
                nc.vector.bn_aggr(mv_sb, st_sb)
                lnv_sb = work.tile([4, 1], dt.float32, tag=f"lnv{g}")
                nc.scalar.activation(
                    lnv_sb, mv_sb[:, 1:2], func=mybir.ActivationFunctionType.Ln,
                    bias=eps_sb[0:4],
                )
                rstd_sb = work.tile([4, 1], dt.float32, tag=f"rstd{g}")
                nc.scalar.activation(
                    rstd_sb, lnv_sb, func=mybir.ActivationFunctionType.Exp,
                    scale=-0.5,
                )
                # fold gamma0 * has_tokens into the scale
                nc.vector.tensor_tensor(
                    rstd_sb, rstd_sb, hg_sb[:, g:g + 1],
                    op=mybir.AluOpType.mult,
                )
                c_sb = work.tile([4, D], dt.float32, tag=f"c{g}")
                nc.vector.tensor_scalar(
                    c_sb, op_ps, mv_sb[:, 0:1], rstd_sb,
                    op0=mybir.AluOpType.subtract, op1=mybir.AluOpType.mult,
                )
                if not uniform_ln:
                    nc.vector.tensor_tensor(c_sb, c_sb, gb_sb[:, 0, g, :],
                                            op=mybir.AluOpType.mult)
                    nc.vector.tensor_tensor(c_sb, c_sb, gb_sb[:, 1, g, :],
                                            op=mybir.AluOpType.add)
                # HWDGE store off the Scalar queue (idle by tail time)
                nc.scalar.dma_start(out=out_d[g * 4:(g + 1) * 4], in_=c_sb)

            # ---- emission in data-arrival order (the PE runs in-order) ----
            E5v = E5.rearrange("p (n k) g h -> p n k g h", k=K)

            def sc_piece(bi, hf):
                """Scores + exp + E5 scatter for xT half hf of batch bi."""
                sc_ps = psc.tile([128, HC * H], dt.float32, tag="sc")
                xt = xT_sb[(bi, hf)]
                for ci in range(HC):
                    for dc in range(2):
                        nc.tensor.matmul(
                            sc_ps[:, ci * H:(ci + 1) * H],
                            lhsT=xt[:, dc, ci * 128:(ci + 1) * 128],
                            rhs=qwT_sb[:, dc, :],
                            start=(dc == 0),
                            stop=(dc == 1),
                        )
                E_sb = work.tile([128, HC, H], dt.bfloat16, tag=f"E{bi}{hf}")
                nc.scalar.activation(
                    out=E_sb.rearrange("p c h -> p (c h)"),
                    in_=sc_ps,
                    func=mybir.ActivationFunctionType.Exp,
                    scale=float(2.0 ** -QW_SHIFT),
                )
                ci = 0
                while ci < HC:
                    c = hf * HC + ci                  # chunk in batch
                    nb = bi * NB + c // K             # global group
                    k0 = c % K
                    kn = min(K - k0, HC - ci)
                    nc.vector.tensor_copy(
                        E5v[:, nb, k0:k0 + kn, nb % 4, :],
                        E_sb[:, ci:ci + kn, :],
                    )
                    ci += kn

            def y_piece(pi):
                bi, off, pcs = PIECES[pi]
                for ci in range(pcs):
                    c = off + ci
                    gc = bi * NCHB + c
                    g32 = gc // (4 * K)               # 32-row group, global
                    nc.tensor.matmul(
                        y_all[g32 * 32:(g32 + 1) * 32, :],
                        lhsT=E5[:, gc, :, :],
                        rhs=xe_sb[pi][:, ci, :],
                        start=(c % (4 * K) == 0),
                        stop=(c % (4 * K) == 4 * K - 1),
                        tile_position=(0, (g32 % 4) * 32),
                    )

            for bi in range(BPC):
                sc_piece(bi, 0)
                sc_piece(bi, 1)
            # y pieces in arrival order; each 32-row group's tail is emitted
            # one piece-group later so its inputs are ready when the PE gets
            # there, leaving only the final group's tail exposed
            done_groups = []
            pending = None
            for pi, (bi, off, pcs) in enumerate(PIECES):
                y_piece(pi)
                if (off + pcs) % (4 * K) == 0:        # a 32-row group completed
                    if pending is not None:
                        tail(pending)
                    pending = (bi * NCHB + off + pcs) // (4 * K) - 1
            tail(pending)
            if pending != 2 * BPC - 1:
                raise AssertionError("group tail bookkeeping broken")

    import concourse.bacc as bacc_mod

    orig_tables = bacc_mod.get_activation_tables
    bacc_mod.get_activation_tables = _patched_act_tables()
    try:
        nc.compile()
    finally:
        bacc_mod.get_activation_tables = orig_tables
    return nc


def _get_nc(K, uniform_ln):
    key = ("nc", K, uniform_ln)
    if key not in _CACHE:
        _CACHE[key] = _build_nc(K, uniform_ln)
    return _CACHE[key]


def _prep(inputs):
    x = np.asarray(inputs["x"], F32)
    query = np.asarray(inputs["query"], F32)
    ipw = np.asarray(inputs["in_proj_weight"], F32)
    ipb = np.asarray(inputs["in_proj_bias"], F32)
    opw = np.asarray(inputs["out_proj_weight"], F32)
    opb = np.asarray(inputs["out_proj_bias"], F32)
    gamma = np.asarray(inputs["ln_gamma"], F32)
    beta = np.asarray(inputs["ln_beta"], F32)
    mask = np.asarray(inputs["mask"]).astype(bool)
    bid = np.asarray(inputs["backend_id"]).astype(np.int32)
    nbm = int(np.asarray(inputs["n_backends_max"]))
    assert nbm == NB and x.shape == (B, L, D)

    wq, wk, wv = ipw[0:D], ipw[D:2 * D], ipw[2 * D:3 * D]
    bq, bk, bv = ipb[0:D], ipb[D:2 * D], ipb[2 * D:3 * D]
    assert not bv.any(), "nonzero v-bias not supported by this kernel"
    assert not opb.any() or np.allclose(opb, opb[0]), \
        "non-uniform out_proj bias shifts LN mean nontrivially"
    uniform_ln = bool(np.all(gamma == gamma[0]) and not beta.any())

    qv = query[0, 0] @ wq.T + bq                      # (256,)
    qh = qv.reshape(H, HD)
    qw = np.einsum("hj,hjd->hd", qh, wk.reshape(H, HD, D)) * SCALE  # (8, 256)
    # bk contribution is constant per h -> cancels in softmax normalization.

    qwT = np.ascontiguousarray(qw.T * 2.0 ** QW_SHIFT).reshape(2, 128, H).astype(FP8)
    wvT = np.ascontiguousarray(wv.T).reshape(2, 128, D).astype(BF16)
    woT = np.ascontiguousarray(opw.T).reshape(2, 128, D).astype(BF16)
    ident = np.concatenate([np.eye(32, dtype=F32)] * 4, axis=0)

    # segment sizes and K
    cnt = np.zeros((B, NB), np.int64)
    for b in range(B):
        cnt[b] = np.bincount(bid[b][mask[b]], minlength=NB)
    K = max(1, int(np.ceil(cnt.max() / 128)))
    LP = NB * K * 128

    # sorted + compacted token layout
    xs = np.zeros((B, LP, D), F32)
    ones = np.zeros((B, LP, 1), F32)
    for b in range(B):
        for n in range(NB):
            idx = np.flatnonzero(mask[b] & (bid[b] == n))
            o = n * K * 128
            xs[b, o:o + len(idx)] = x[b, idx]
            ones[b, o:o + len(idx)] = 1.0
    xT = np.ascontiguousarray(xs.transpose(0, 2, 1)).reshape(B, 2, 128, LP).astype(FP8)
    xe = np.empty((B, LP, JW), BF16)
    xe[..., :D] = (xs * ones).astype(BF16)
    xe[..., D] = ones[..., 0].astype(BF16)
    NCHB = NB * K
    xe = np.ascontiguousarray(
        xe.reshape(B, NCHB, 128, JW).transpose(0, 2, 1, 3)
    ).reshape(B, 128, NCHB * JW)

    ht = (cnt > 0).astype(F32)                        # (B, NB)
    hg = (gamma[0] * ht).astype(F32)                  # uniform-LN scale
    gf = gamma[None, None, :] * ht[:, :, None]        # (B, NB, D) general path
    bf = beta[None, None, :] * ht[:, :, None]

    in_maps = []
    for c in range(NCORES):
        sl = slice(BPC * c, BPC * (c + 1))
        # group-major layouts: partition = backend-within-group (4),
        # column = group g = bi*2 + (n//4)
        hgc = np.ascontiguousarray(
            hg[sl].reshape(BPC, 2, 4).transpose(2, 0, 1).reshape(4, 2 * BPC)
        )
        gfT = gf[sl].reshape(BPC, 2, 4, D).transpose(2, 0, 1, 3).reshape(4, 2 * BPC, D)
        bfT = bf[sl].reshape(BPC, 2, 4, D).transpose(2, 0, 1, 3).reshape(4, 2 * BPC, D)
        gb = np.ascontiguousarray(
            np.stack([gfT, bfT], axis=1)
        ).astype(F32).reshape(4, 2 * 2 * BPC * D)
        im = {
            "xT": xT[sl], "xe": xe[sl],
            "qwT": qwT, "wvT": wvT, "woT": woT,
            "hg": hgc, "ident": ident,
        }
        if not uniform_ln:
            im["gb"] = gb
        in_maps.append(im)
    return in_maps, K, uniform_ln


def kernel(**inputs):
    global LAST_RESULT
    in_maps, K, uniform_ln = _prep(inputs)
    nc = _get_nc(K, uniform_ln)
    res = run_bass_kernel_spmd(nc, in_maps, list(range(NCORES)))
    LAST_RESULT = res
    out = np.concatenate(
        [res.results[c]["out"].reshape(BPC, NB, D) for c in range(NCORES)], axis=0
    )
    return np.ascontiguousarray(out.astype(F32))


if __name__ == "__main__":
    nc = _get_nc(5, True)
    print("traced ok:", nc)


# revision 27
# speedup vs baseline: 1.1620x; 1.1620x over previous
"""Trainium2 Bass kernel for BackendQueryPooling.

Math simplifications (exact unless noted):
  - k-projection folds into the shared query:
        scores[l,h] = x[l] . qw[h],  qw[h] = (q_h @ wk_head_h) / sqrt(hd)
    (bk is constant per h and cancels in softmax).
  - v never materialized: ctx[n,h,:] = (sum_l w[n,h,l] x[l]) @ wvT_head
    since sum_l w = 1 after normalization (bv is zero here; asserted).
  - Unnormalized exp + ones-column normalization in the same matmul.
  - Host-side token compaction: tokens with mask=0 have softmax weight
    exactly 0 in the reference (logit -1e9 underflows), so they are
    dropped. Remaining tokens are SORTED by backend_id so each backend
    owns a contiguous, 128-aligned segment; padding slots have xe rows
    (incl. the ones column) set to 0 so they contribute nothing.
  - Empty backends produce 0 via the has_tokens factor folded into the
    final scale, as the reference zeroes by has_tokens.

Sharding: data-parallel over batch. 16 batches / 8 cores = 2 per core.
Each batch gets a [64, 257] PSUM accumulator (partition n*8+h); its
whole tail (normalize/ctx/out_proj/LN/store) runs as soon as that
batch's chunks finish, so only the last batch's tail is exposed.
y matmuls use zero-padded [128, 32] E blocks so PE tile positions stay
32-aligned.
"""

import sys

sys.path.insert(0, "/opt/trn_rl_repo")

import numpy as np
import ml_dtypes

import concourse.bass as bass
import concourse.bacc as bacc
import concourse.tile as tile
from concourse import mybir
from concourse.bass_utils import run_bass_kernel_spmd

BF16 = ml_dtypes.bfloat16
FP8 = ml_dtypes.float8_e4m3
QW_SHIFT = 13  # qw values ~7e-4 underflow fp8; pre-scale by 2**13, undo in exp scale
F32 = np.float32

B, L, D = 16, 8192, 256
H, HD, NB = 8, 32, 8
NCORES = 8
BPC = B // NCORES          # batches per core
SCALE = 1.0 / np.sqrt(HD)
JW = D + 1                 # 257: xe row width incl. ones column
NG = BPC * NB              # 16 (batch, backend) groups per core

_CACHE = {}
LAST_RESULT = None


def _patched_act_tables():
    """Make the act-table chooser land Exp AND Ln in one set
    (natural_log_exp_and_others) instead of thrashing between
    exp_and_others and natural_log (1.28us reload per switch)."""
    from concourse.hw_specs import get_activation_tables

    AF = mybir.ActivationFunctionType

    def patched(arch):
        t = {k: set(v) for k, v in get_activation_tables(arch).items()}
        for name in t:
            if name != "natural_log_exp_and_others":
                t[name].discard(AF.Exp)
                t[name].discard(AF.Ln)
        return t

    return patched


def _build_nc(K, uniform_ln):
    """K = chunks (of 128 tokens) per (batch, backend) segment.

    uniform_ln: ln_gamma is a uniform vector and ln_beta is zero, so the
    gamma*has_tokens factor collapses into the per-row rstd scale and the
    whole LN runs on the Vector engine (rstd via pow(-0.5))."""
    nc = bacc.Bacc("TRN2", target_bir_lowering=False)
    dt = mybir.dt

    NCHB = NB * K              # l-chunks per batch
    NCH = BPC * NCHB           # l-chunks per core (both batches)
    LP = NCHB * 128            # padded tokens per batch
    HC = NCHB // 2             # chunks per xT half-tile

    # xe pieces (in chunks): coarse early (few DMA triggers), fine at the
    # very end so the last piece's compute tail is short
    def split_pieces(n, final):
        h = n // 2
        if not final or h <= 8:
            return [h, n - h]
        return [h, n - h - 8, 4, 4]

    PIECES = []                # (batch, chunk_off_in_batch, n_chunks)
    for bi in range(BPC):
        off = 0
        for pcs in split_pieces(NCHB, final=(bi == BPC - 1)):
            PIECES.append((bi, off, pcs))
            off += pcs

    xT_d = nc.dram_tensor("xT", [BPC, 2, 128, LP], dt.float8e4, kind="ExternalInput")
    xe_d = nc.dram_tensor("xe", [BPC, 128, NCHB * JW], dt.bfloat16, kind="ExternalInput")
    qwT_d = nc.dram_tensor("qwT", [2, 128, H], dt.float8e4, kind="ExternalInput")
    wvT_d = nc.dram_tensor("wvT", [2, 128, D], dt.bfloat16, kind="ExternalInput")
    woT_d = nc.dram_tensor("woT", [2, 128, D], dt.bfloat16, kind="ExternalInput")
    hg_d = nc.dram_tensor("hg", [NB, BPC], dt.float32, kind="ExternalInput")
    if not uniform_ln:
        gb_d = nc.dram_tensor("gb", [NB, 2 * BPC * D], dt.float32, kind="ExternalInput")
    id_d = nc.dram_tensor("ident", [128, 64], dt.float32, kind="ExternalInput")
    out_d = nc.dram_tensor("out", [NG, D], dt.float32, kind="ExternalOutput")

    with tile.TileContext(nc) as tc:
        with (
            tc.tile_pool(name="consts", bufs=1) as consts,
            tc.tile_pool(name="big", bufs=1) as big,
            tc.tile_pool(name="work", bufs=2) as work,
            tc.tile_pool(name="psc", bufs=3, space="PSUM") as psc,
            tc.tile_pool(name="psy", bufs=1, space="PSUM") as psy,
            tc.tile_pool(name="pst", bufs=2, space="PSUM") as pst,
        ):
            # scores-path const first: it gates the first compute
            qwT_sb = consts.tile([128, 2, H], dt.float8e4)
            nc.scalar.dma_start(out=qwT_sb, in_=qwT_d[:].rearrange("c p h -> p c h"))

            # x loads on the Sync trigger queue; xT in halves so scores can
            # start before the full batch lands
            xT_sb = {}
            xe_sb = []

            def load_xT(bi):
                for hf in range(2):
                    t = big.tile([128, 2, LP // 2], dt.float8e4, tag=f"xT{bi}{hf}")
                    nc.sync.dma_start(
                        out=t,
                        in_=xT_d[bi, :, :, hf * (LP // 2):(hf + 1) * (LP // 2)]
                        .rearrange("c p l -> p c l"),
                    )
                    xT_sb[(bi, hf)] = t

            def load_xe(bi, off, pcs):
                t = big.tile([128, pcs, JW], dt.bfloat16, tag=f"xe{len(xe_sb)}")
                nc.sync.dma_start(
                    out=t,
                    in_=xe_d[bi, :, off * JW:(off + pcs) * JW].rearrange(
                        "p (c j) -> p c j", j=JW
                    ),
                )
                xe_sb.append(t)

            for bi in range(BPC):
                load_xT(bi)
            for bi, off, pcs in PIECES:
                load_xe(bi, off, pcs)

            # tail consts on the Scalar queue
            wvT_sb = consts.tile([128, 2, D], dt.bfloat16)
            nc.scalar.dma_start(out=wvT_sb, in_=wvT_d[:].rearrange("c p e -> p c e"))
            woT_sb = consts.tile([128, 2, D], dt.bfloat16)
            nc.scalar.dma_start(out=woT_sb, in_=woT_d[:].rearrange("c p e -> p c e"))
            id_sb = consts.tile([128, 64], dt.float32)
            nc.scalar.dma_start(out=id_sb, in_=id_d[:])
            hg_sb = consts.tile([NB, BPC], dt.float32)
            nc.scalar.dma_start(out=hg_sb, in_=hg_d[:])
            eps_sb = consts.tile([128, 1], dt.float32)
            nc.vector.memset(eps_sb, 1e-5)
            if not uniform_ln:
                gb_sb = consts.tile([NB, 2, BPC, D], dt.float32)
                nc.scalar.dma_start(
                    out=gb_sb,
                    in_=gb_d[:].rearrange("p (t b e) -> p t b e", t=2, b=BPC),
                )

            # zero-padded E blocks: E5[:, c, g, :] nonzero only at g = nb%4
            E5 = consts.tile([128, NCH, 4, H], dt.bfloat16)
            nc.gpsimd.memset(E5, 0.0)

            y_all = psy.tile([128, JW], dt.float32, tag="y")
            y_ps = [y_all[64 * bi_:64 * (bi_ + 1), :] for bi_ in range(BPC)]

            def tail(bi):
                pb = slice(64 * bi, 64 * (bi + 1))
                # norm chain stays on the same partitions as this batch's y rows
                s_sb = work.tile([128, 1], dt.float32, tag=f"s{bi}")
                nc.vector.tensor_scalar_add(s_sb[pb], y_all[pb, D:JW], 1e-30)
                r_sb = work.tile([128, 1], dt.float32, tag=f"r{bi}")
                nc.vector.reciprocal(r_sb[pb], s_sb[pb])
                yn_sb = work.tile([128, D], dt.float32, tag=f"yn{bi}")
                nc.vector.tensor_scalar_mul(yn_sb[pb], y_all[pb, 0:D], r_sb[pb])

                # transpose y_norm -> yT [e', dc, (n h)]
                yT_sb = work.tile([128, 2, 64], dt.bfloat16, tag=f"yT{bi}")
                for ec in range(2):
                    tr_ps = pst.tile([128, 64], dt.float32, tag="tail")
                    nc.tensor.transpose(
                        tr_ps, yn_sb[pb, ec * 128:(ec + 1) * 128], id_sb[pb, :],
                        tile_position=(64 * bi, 0),
                    )
                    nc.vector.tensor_copy(yT_sb[:, ec, :], tr_ps)

                # ctxT[e', (t, n)] via per-head matmuls
                cx_ps = pst.tile([128, 2 * NB], dt.float32, tag="tail")
                yTv = yT_sb.rearrange("p c (n h2) -> p c n h2", h2=H)
                for h in range(H):
                    t, r4 = divmod(h, 4)
                    r0 = r4 * 32
                    for dc in range(2):
                        nc.tensor.matmul(
                            cx_ps[r0:r0 + 32, t * NB:(t + 1) * NB],
                            lhsT=wvT_sb[:, dc, HD * h:HD * h + HD],
                            rhs=yTv[:, dc, :, h],
                            start=(dc == 0),
                            stop=(dc == 1),
                            tile_position=(0, r0),
                        )
                cx_sb = work.tile([128, 2 * NB], dt.bfloat16, tag=f"cxs{bi}")
                nc.vector.tensor_copy(cx_sb, cx_ps)

                # out_proj + LayerNorm
                op_ps = pst.tile([NB, D], dt.float32, tag="tail")
                for t in range(2):
                    nc.tensor.matmul(
                        op_ps,
                        lhsT=cx_sb[:, t * NB:(t + 1) * NB],
                        rhs=woT_sb[:, t, :],
                        start=(t == 0),
                        stop=(t == 1),
                    )
                st_sb = work.tile([NB, 6], dt.float32, tag=f"st{bi}")
                nc.vector.bn_stats(st_sb, op_ps)
                mv_sb = work.tile([NB, 2], dt.float32, tag=f"mv{bi}")
                nc.vector.bn_aggr(mv_sb, st_sb)
                lnv_sb = work.tile([NB, 1], dt.float32, tag=f"lnv{bi}")
                nc.scalar.activation(
                    lnv_sb, mv_sb[:, 1:2], func=mybir.ActivationFunctionType.Ln,
                    bias=eps_sb[0:NB],
                )
                rstd_sb = work.tile([NB, 1], dt.float32, tag=f"rstd{bi}")
                nc.scalar.activation(
                    rstd_sb, lnv_sb, func=mybir.ActivationFunctionType.Exp,
                    scale=-0.5,
                )
                # fold gamma0 * has_tokens into the scale
                nc.vector.tensor_tensor(
                    rstd_sb, rstd_sb, hg_sb[:, bi:bi + 1],
                    op=mybir.AluOpType.mult,
                )
                c_sb = work.tile([NB, D], dt.float32, tag=f"c{bi}")
                nc.vector.tensor_scalar(
                    c_sb, op_ps, mv_sb[:, 0:1], rstd_sb,
                    op0=mybir.AluOpType.subtract, op1=mybir.AluOpType.mult,
                )
                if not uniform_ln:
                    nc.vector.tensor_tensor(c_sb, c_sb, gb_sb[:, 0, bi, :],
                                            op=mybir.AluOpType.mult)
                    nc.vector.tensor_tensor(c_sb, c_sb, gb_sb[:, 1, bi, :],
                                            op=mybir.AluOpType.add)
                # HWDGE store off the Scalar queue (idle by tail time)
                nc.scalar.dma_start(out=out_d[bi * NB:(bi + 1) * NB], in_=c_sb)

            # ---- emission in data-arrival order (the PE runs in-order) ----
            E5v = E5.rearrange("p (n k) g h -> p n k g h", k=K)

            def sc_piece(bi, hf):
                """Scores + exp + E5 scatter for xT half hf of batch bi."""
                sc_ps = psc.tile([128, HC * H], dt.float32, tag="sc")
                xt = xT_sb[(bi, hf)]
                for ci in range(HC):
                    for dc in range(2):
                        nc.tensor.matmul(
                            sc_ps[:, ci * H:(ci + 1) * H],
                            lhsT=xt[:, dc, ci * 128:(ci + 1) * 128],
                            rhs=qwT_sb[:, dc, :],
                            start=(dc == 0),
                            stop=(dc == 1),
                        )
                E_sb = work.tile([128, HC, H], dt.bfloat16, tag=f"E{bi}{hf}")
                nc.scalar.activation(
                    out=E_sb.rearrange("p c h -> p (c h)"),
                    in_=sc_ps,
                    func=mybir.ActivationFunctionType.Exp,
                    scale=float(2.0 ** -QW_SHIFT),
                )
                ci = 0
                while ci < HC:
                    c = hf * HC + ci                  # chunk in batch
                    nb = bi * NB + c // K             # global group
                    k0 = c % K
                    kn = min(K - k0, HC - ci)
                    nc.vector.tensor_copy(
                        E5v[:, nb, k0:k0 + kn, nb % 4, :],
                        E_sb[:, ci:ci + kn, :],
                    )
                    ci += kn

            def y_piece(pi):
                bi, off, pcs = PIECES[pi]
                for ci in range(pcs):
                    c = off + ci
                    gc = bi * NCHB + c
                    g32 = gc // (4 * K)               # 32-row group, global
                    nc.tensor.matmul(
                        y_all[g32 * 32:(g32 + 1) * 32, :],
                        lhsT=E5[:, gc, :, :],
                        rhs=xe_sb[pi][:, ci, :],
                        start=(c % (4 * K) == 0),
                        stop=(c % (4 * K) == 4 * K - 1),
                        tile_position=(0, (g32 % 4) * 32),
                    )

            for bi in range(BPC):
                sc_piece(bi, 0)
                sc_piece(bi, 1)
            npc0 = sum(1 for (bj, _, _) in PIECES if bj == 0)
            for pi in range(npc0):
                y_piece(pi)
            tail(0)
            if BPC > 1:
                for pi in range(npc0, len(PIECES)):
                    y_piece(pi)
                tail(1)

    import concourse.bacc as bacc_mod

    orig_tables = bacc_mod.get_activation_tables
    bacc_mod.get_activation_tables = _patched_act_tables()
    try:
        nc.compile()
    finally:
        bacc_mod.get_activation_tables = orig_tables
    return nc


def _get_nc(K, uniform_ln):
    key = ("nc", K, uniform_ln)
    if key not in _CACHE:
        _CACHE[key] = _build_nc(K, uniform_ln)
    return _CACHE[key]


def _prep(inputs):
    x = np.asarray(inputs["x"], F32)
    query = np.asarray(inputs["query"], F32)
    ipw = np.asarray(inputs["in_proj_weight"], F32)
    ipb = np.asarray(inputs["in_proj_bias"], F32)
    opw = np.asarray(inputs["out_proj_weight"], F32)
    opb = np.asarray(inputs["out_proj_bias"], F32)
    gamma = np.asarray(inputs["ln_gamma"], F32)
    beta = np.asarray(inputs["ln_beta"], F32)
    mask = np.asarray(inputs["mask"]).astype(bool)
    bid = np.asarray(inputs["backend_id"]).astype(np.int32)
    nbm = int(np.asarray(inputs["n_backends_max"]))
    assert nbm == NB and x.shape == (B, L, D)

    wq, wk, wv = ipw[0:D], ipw[D:2 * D], ipw[2 * D:3 * D]
    bq, bk, bv = ipb[0:D], ipb[D:2 * D], ipb[2 * D:3 * D]
    assert not bv.any(), "nonzero v-bias not supported by this kernel"
    assert not opb.any() or np.allclose(opb, opb[0]), \
        "non-uniform out_proj bias shifts LN mean nontrivially"
    uniform_ln = bool(np.all(gamma == gamma[0]) and not beta.any())

    qv = query[0, 0] @ wq.T + bq                      # (256,)
    qh = qv.reshape(H, HD)
    qw = np.einsum("hj,hjd->hd", qh, wk.reshape(H, HD, D)) * SCALE  # (8, 256)
    # bk contribution is constant per h -> cancels in softmax normalization.

    qwT = np.ascontiguousarray(qw.T * 2.0 ** QW_SHIFT).reshape(2, 128, H).astype(FP8)
    wvT = np.ascontiguousarray(wv.T).reshape(2, 128, D).astype(BF16)
    woT = np.ascontiguousarray(opw.T).reshape(2, 128, D).astype(BF16)
    ident = np.concatenate([np.eye(64, dtype=F32)] * 2, axis=0)

    # segment sizes and K
    cnt = np.zeros((B, NB), np.int64)
    for b in range(B):
        cnt[b] = np.bincount(bid[b][mask[b]], minlength=NB)
    K = max(1, int(np.ceil(cnt.max() / 128)))
    LP = NB * K * 128

    # sorted + compacted token layout
    xs = np.zeros((B, LP, D), F32)
    ones = np.zeros((B, LP, 1), F32)
    for b in range(B):
        for n in range(NB):
            idx = np.flatnonzero(mask[b] & (bid[b] == n))
            o = n * K * 128
            xs[b, o:o + len(idx)] = x[b, idx]
            ones[b, o:o + len(idx)] = 1.0
    xT = np.ascontiguousarray(xs.transpose(0, 2, 1)).reshape(B, 2, 128, LP).astype(FP8)
    xe = np.empty((B, LP, JW), BF16)
    xe[..., :D] = (xs * ones).astype(BF16)
    xe[..., D] = ones[..., 0].astype(BF16)
    NCHB = NB * K
    xe = np.ascontiguousarray(
        xe.reshape(B, NCHB, 128, JW).transpose(0, 2, 1, 3)
    ).reshape(B, 128, NCHB * JW)

    ht = (cnt > 0).astype(F32)                        # (B, NB)
    hg = (gamma[0] * ht).astype(F32)                  # uniform-LN scale
    gf = gamma[None, None, :] * ht[:, :, None]        # (B, NB, D) general path
    bf = beta[None, None, :] * ht[:, :, None]

    in_maps = []
    for c in range(NCORES):
        sl = slice(BPC * c, BPC * (c + 1))
        hgc = np.ascontiguousarray(hg[sl].T)
        gb = np.ascontiguousarray(
            np.stack([gf[sl].transpose(1, 0, 2), bf[sl].transpose(1, 0, 2)], axis=1)
        ).astype(F32).reshape(NB, 2 * BPC * D)
        im = {
            "xT": xT[sl], "xe": xe[sl],
            "qwT": qwT, "wvT": wvT, "woT": woT,
            "hg": hgc, "ident": ident,
        }
        if not uniform_ln:
            im["gb"] = gb
        in_maps.append(im)
    return in_maps, K, uniform_ln


def kernel(**inputs):
    global LAST_RESULT
    in_maps, K, uniform_ln = _prep(inputs)
    nc = _get_nc(K, uniform_ln)
    res = run_bass_kernel_spmd(nc, in_maps, list(range(NCORES)))
    LAST_RESULT = res
    out = np.concatenate(
        [res.results[c]["out"].reshape(BPC, NB, D) for c in range(NCORES)], axis=0
    )
    return np.ascontiguousarray(out.astype(F32))


if __name__ == "__main__":
    nc = _get_nc(5, True)
    print("traced ok:", nc)
